# revision 42
# baseline (speedup 1.0000x reference)
"""Trainium2 Bass kernel for nn_MultiHeadAttention (channel-attention transformer block).

Math (per batch b, with X* = reshape(*, [C, P]), P = 4096, C = 128, D = 512):
  Q = Xq @ (Wq/temp)^T, K = Xk @ Wk^T, V = Xv @ Wv^T            [C, D]
  per head h (8 heads, ld=64): A_h = softmax(Q_h K_h^T); O_h = A_h V_h
  O = silu(O); O = (O - mean)/(unbiased_std + eps)   (LN affine folded into fc)
  out_pre = (v + Wfc@ln_beta) + O @ (Wfc*ln_gamma)^T
  out = BatchNorm2d(out_pre)   (batch stats over (b,h,w), biased var)

Sharding: data-parallel over batch, 2 batches per core on 8 cores; BatchNorm
statistics combined with a tiny AllReduce ([128,2] per core).

Design notes:
  - weights quantized to fp8 E3M4 with power-of-2 scales folded into existing
    per-row scalars (exp scale, sigmoid scale, LN-sqrt scale) => zero extra ops
  - activations/outputs in bf16 (DMA ~19MB/core vs 50MB f32); everything is
    prefetched to SBUF via contiguous quarter-major DMAs
  - Q/K projections weight-stationary with both batches concatenated into one
    256-wide moving operand => QT/KT land pre-transposed, one PSUM group/bank
  - attention via the ST trick: ST_h = K_h Q_h^T, exp(ST) = A^T is directly
    the AV stationary operand (no PE transposes / PSUM shuffles); softmax
    denominators via a 1-wide matmul against a ones vector
  - ACT table grouping (all exps, then sigmoids, then sqrts)
  - BN partial sums via accum_out on the residual-add (DVE) + Square (ACT)
  - bf16 output, upcast on host; BN stats via one [128,2] AllReduce

HW-crash landmine (cost a lot of bisection): matmul/transpose outputs into
PSUM must be plain 2D slices of tiles that fit ONE 2KB bank, and a psum tile
may hold only one OPEN accumulation group at a time. 3D-sliced outs
(tile[:, h, :]) or multi-bank out tiles crash the device (NRT UNRECOVERABLE).
"""

import os

import numpy as np
import ml_dtypes

import concourse.mybir as mybir
import concourse.tile as tile
from concourse import bacc
from concourse.bass_utils import run_bass_kernel_spmd
from concourse.masks import make_identity

# ---- problem constants (hardcoded per contract) ----
B, C, HH, WW = 16, 128, 64, 64
P = HH * WW           # 4096
NH, LD = 8, 64
D = NH * LD           # 512
N_CORES = 8
BPC = B // N_CORES    # 2 batches per core
NCH = P // 128        # 32 pixel chunks (contraction)
NPT = P // 512        # 8 output column tiles for fc
LN_EPS = 1e-6
BN_EPS = 1e-5
F32 = mybir.dt.float32
BF16 = mybir.dt.bfloat16
W8MODE = os.environ.get("BASS_W8", "e3")  # e3 | e4 | bf16
FP8 = {"e3": mybir.dt.float8e3, "e4": mybir.dt.float8e4,
       "bf16": mybir.dt.bfloat16}[W8MODE]
W8BYTES = 2 if W8MODE == "bf16" else 1
FP8_MAX_TARGET = {"e3": 14.0, "e4": 224.0, "bf16": 14.0}[W8MODE]

_BUILD_CACHE: dict = {}
LAST_RESULTS = None  # BassKernelResults of the most recent run (for profiling)

# host-side fp8 scales (power of two), computed at pack time, baked into build
_SCALES: dict = {}


def _emit(ctx, nc, tc, io, scales):
    PH = int(os.environ.get("BASS_PHASES", "9"))
    AF = mybir.ActivationFunctionType
    ALU = mybir.AluOpType
    AX = mybir.AxisListType
    s_q, s_k, s_v, s_fc = (scales[k] for k in ("s_q", "s_k", "s_v", "s_fc"))

    consts = ctx.enter_context(tc.tile_pool(name="consts", bufs=1))
    big = ctx.enter_context(tc.tile_pool(name="big", bufs=1))
    sb = ctx.enter_context(tc.tile_pool(name="sb", bufs=2))
    small = ctx.enter_context(tc.tile_pool(name="small", bufs=4))
    stat = ctx.enter_context(tc.tile_pool(name="stat", bufs=1))
    dram = ctx.enter_context(tc.tile_pool(name="dram", bufs=1, space="DRAM"))

    ident = consts.tile([128, 128], BF16, tag="ident", name="ident")
    identf = consts.tile([128, 128], F32, tag="identf", name="identf")
    make_identity(nc, identf)
    nc.vector.tensor_copy(out=ident, in_=identf)
    ones = consts.tile([128, 1], BF16, tag="ones", name="ones")
    nc.vector.memset(ones, 1.0)

    bng = consts.tile([128, 1], F32, tag="bng", name="bng")
    bnb = consts.tile([128, 1], F32, tag="bnb", name="bnb")
    epsbn = consts.tile([128, 1], F32, tag="epsbn", name="epsbn")
    nc.gpsimd.dma_start(out=bng, in_=io["bng"][:, :])
    nc.gpsimd.dma_start(out=bnb, in_=io["bnb"][:, :])
    nc.vector.memset(epsbn, BN_EPS)

    # ---- prefetch: everything lives in SBUF (quartered DMAs so compute can
    # start as soon as the first chunks land)
    qa_sb = big.tile([128, NCH, 2, 128], BF16, tag="qa_sb", name="qa_sb")
    ka_sb = big.tile([128, NCH, 2, 128], BF16, tag="ka_sb", name="ka_sb")
    va_sb = big.tile([128, NCH, 2, 128], BF16, tag="va_sb", name="va_sb")
    wq_sb = big.tile([128, NCH, 4, 128], FP8, tag="wq_sb", name="wq_sb")
    wk_sb = big.tile([128, NCH, 4, 128], FP8, tag="wk_sb", name="wk_sb")
    wv_sb = big.tile([128, NCH, 512], FP8, tag="wv_sb", name="wv_sb")
    QTR = NCH // 4
    for q4 in range(4):
        cs = slice(QTR * q4, QTR * (q4 + 1))
        nc.sync.dma_start(out=qa_sb[:, cs, :, :], in_=io["qa"][q4])
        nc.scalar.dma_start(out=wq_sb[:, cs, :, :], in_=io["wq"][q4])
    for q4 in range(4):
        cs = slice(QTR * q4, QTR * (q4 + 1))
        nc.sync.dma_start(out=ka_sb[:, cs, :, :], in_=io["ka"][q4])
        nc.scalar.dma_start(out=wk_sb[:, cs, :, :], in_=io["wk"][q4])
    for q4 in range(4):
        cs = slice(QTR * q4, QTR * (q4 + 1))
        nc.sync.dma_start(out=va_sb[:, cs, :, :], in_=io["va"][q4])
        nc.scalar.dma_start(out=wv_sb[:, cs, :], in_=io["wv"][q4])

    # residual (+ folded fc bias) and fc weights queue on sync BEHIND the
    # phase-A activation feeds so they don't steal HBM bandwidth early
    veff_sb = []
    for b in range(BPC):
        t = big.tile([128, P], BF16, tag=f"veff{b}", name=f"veff{b}")
        nc.sync.dma_start(out=t, in_=io["veff"][b, :, :])
        veff_sb.append(t)
    wfc_sb = big.tile([128, NPT, 4, 512], FP8, tag="wfc_sb", name="wfc_sb")
    nc.sync.dma_start(out=wfc_sb, in_=io["wfc"])
    out_sb = [big.tile([128, P], BF16, tag=f"outb{b}", name=f"outb{b}")
              for b in range(BPC)]

    # ---- phase A: QKV projections, accumulating over the P=4096 contraction.
    # Q,K weight-stationary with both batches as one 256-wide moving operand
    # (outputs arrive transposed [d, b|c]); V activation-stationary 512-wide.
    ctx_a1 = tc.tile_pool(name="ps_qk", bufs=3, space="PSUM")
    ps_qk = ctx_a1.__enter__()
    ctx_a2 = tc.tile_pool(name="ps_v", bufs=1, space="PSUM")
    ps_v = ctx_a2.__enter__()
    warm = ps_v.tile([128, 128], BF16, tag="warm", name="warm")
    for _ in range(40):
        nc.tensor.transpose(warm[:, :], ident[:, :], ident[:, :])

    qkv_sb = []
    for b in range(BPC):
        QT_sb = sb.tile([128, 512], BF16, tag=f"QT_sb{b}", name=f"QT_sb{b}")
        KT_sb = sb.tile([128, 512], BF16, tag=f"KT_sb{b}", name=f"KT_sb{b}")
        V_sb = sb.tile([128, 512], BF16, tag=f"V_sb{b}", name=f"V_sb{b}")
        qkv_sb.append((QT_sb, KT_sb, V_sb))

    if PH >= 2:
        for w_sb, a_sb, which in ((wq_sb, qa_sb, 0), (wk_sb, ka_sb, 1)):
            for dc in range(4):
                pq = ps_qk.tile([128, 256], F32, tag="qkp", name="qkp")
                for chunk in range(NCH):
                    nc.tensor.matmul(pq[:, :], w_sb[:, chunk, dc, :],
                                     a_sb[:, chunk, :, :],
                                     start=chunk == 0, stop=chunk == NCH - 1)
                fo = dc * 128
                dst0 = qkv_sb[0][which]
                dst1 = qkv_sb[1][which]
                nc.scalar.copy(out=dst0[:, fo:fo + 128], in_=pq[:, 0:128])
                nc.scalar.copy(out=dst1[:, fo:fo + 128], in_=pq[:, 128:256])
        for b in range(BPC):
            vp = ps_v.tile([128, 512], F32, tag=f"vp{b}", name=f"vp{b}")
            for chunk in range(NCH):
                nc.tensor.matmul(vp[:, :], va_sb[:, chunk, b, :],
                                 wv_sb[:, chunk, :],
                                 start=chunk == 0, stop=chunk == NCH - 1)
            nc.scalar.copy(out=qkv_sb[b][2], in_=vp[:, :])
    ctx_a2.__exit__(None, None, None)
    ctx_a1.__exit__(None, None, None)

    ps_s = ctx.enter_context(tc.tile_pool(name="ps_s", bufs=2, space="PSUM"))
    ps_xt = ctx.enter_context(tc.tile_pool(name="ps_xt", bufs=1, space="PSUM"))
    ps_o = ctx.enter_context(tc.tile_pool(name="ps_o", bufs=1, space="PSUM"))
    ps_fc = ctx.enter_context(tc.tile_pool(name="ps_fc", bufs=2, space="PSUM"))

    # per-channel partial sums: cols 0..15 = sum(out) per (b,pt), 16..31 = sum(out^2)
    pcols = stat.tile([128, 32], F32, tag="pcols", name="pcols")

    exp_scale = 1.0 / (s_q * s_k)
    sig_scale = 1.0 / s_v
    sqrt_scale = (float(D) / (D - 1)) * s_fc * s_fc
    eps_s = LN_EPS * s_v * s_fc

    dbg = os.environ.get("BASS_DEBUG_DUMP", "0") == "1" and "dbg_qt" in io
    if dbg:
        for b in range(BPC):
            nc.gpsimd.dma_start(out=io["dbg_qt"][b], in_=qkv_sb[b][0][:, :])
            nc.gpsimd.dma_start(out=io["dbg_kt"][b], in_=qkv_sb[b][1][:, :])
            nc.gpsimd.dma_start(out=io["dbg_v"][b], in_=qkv_sb[b][2][:, :])

    if PH < 3:
        for b in range(BPC):
            nc.vector.memset(out_sb[b], 0.0)
            eng = nc.sync if b == 0 else nc.scalar
            eng.dma_start(out=io["out"][b, :, :], in_=out_sb[b][:, :])
        return

    # ---- phase B: attention, ST formulation. ST_h = K_h Q_h^T comes out
    # [e, c]; exp(ST) is A^T which is exactly the AV stationary operand, so no
    # PE transposes or PSUM->SBUF shuffles are needed. Softmax denominators
    # come from a 1-wide matmul against a ones vector (same stationary).
    # All exps are contiguous so the ACT Exp table loads once.
    ls = ps_o.tile([128, 16], F32, tag="ls", name="ls")
    warm2 = ps_xt.tile([128, 128], BF16, tag="xtp", name="warm2")
    eftp = ctx.enter_context(tc.tile_pool(name="eftp", bufs=16))
    Ops = []
    efts = {}
    for b in range(BPC):
        QT_sb, KT_sb, V_sb = qkv_sb[b]
        for h in range(NH):
            po = (h % 2) * 64
            fo = (h // 2) * 128
            ST = ps_s.tile([128, 128], F32, tag="S", name="S")
            nc.tensor.matmul(ST[:, :], KT_sb[po:po + 64, fo:fo + 128],
                             QT_sb[po:po + 64, fo:fo + 128], start=True, stop=True)
            eft = eftp.tile([128, 128], BF16, tag="eft", name="eft")
            nc.scalar.activation(out=eft, in_=ST[:, :], func=AF.Exp,
                                 scale=exp_scale)
            efts[(b, h)] = eft
    Oscs = []
    for b in range(BPC):
        V_sb = qkv_sb[b][2]
        Opsum = ps_o.tile([128, 512], F32, tag=f"O{b}", name=f"O{b}")
        Ops.append(Opsum)
        for h in range(NH):
            nc.tensor.matmul(Opsum[:, h * 64:(h + 1) * 64], efts[(b, h)][:, :],
                             V_sb[:, h * 64:(h + 1) * 64], start=True, stop=True)
            nc.tensor.matmul(ls[:, b * 8 + h:b * 8 + h + 1], efts[(b, h)][:, :],
                             ones[:, :], start=True, stop=True)

    for b in range(BPC):
        rs = small.tile([128, 8], F32, tag="rs", name="rs")
        nc.vector.reciprocal(rs, ls[:, b * 8:b * 8 + 8])
        Osc = sb.tile([128, 512], F32, tag=f"Osc{b}", name=f"Osc{b}")
        for h in range(NH):
            nc.scalar.activation(out=Osc[:, h * 64:(h + 1) * 64],
                                 in_=Ops[b][:, h * 64:(h + 1) * 64],
                                 func=AF.Copy, scale=rs[:, h:h + 1])
        Oscs.append(Osc)
        if dbg:
            nc.gpsimd.dma_start(out=io["dbg_osc"][b], in_=Osc[:, :])

    if PH < 4:
        for b in range(BPC):
            nc.vector.memset(out_sb[b], 0.0)
            eng = nc.sync if b == 0 else nc.scalar
            eng.dma_start(out=io["out"][b, :, :], in_=out_sb[b][:, :])
        return

    # ---- phase C: silu + LN (sigmoids grouped, then sqrts, for table reuse)
    Osws = []
    mvs = []
    for b in range(BPC):
        sg = sb.tile([128, D], F32, tag=f"sg{b}", name=f"sg{b}")
        nc.scalar.activation(out=sg, in_=Oscs[b], func=AF.Sigmoid, scale=sig_scale)
        Osw = sb.tile([128, D], F32, tag=f"Osw{b}", name=f"Osw{b}")
        nc.gpsimd.tensor_mul(out=Osw, in0=Oscs[b], in1=sg)
        st6 = small.tile([128, 6], F32, tag="st6", name="st6")
        nc.vector.bn_stats(out=st6, in_=Osw)
        mv = small.tile([128, 2], F32, tag=f"mv{b}", name=f"mv{b}")
        nc.vector.bn_aggr(out=mv, in_=st6)
        Osws.append(Osw)
        mvs.append(mv)
    for _ in range(24):
        nc.tensor.transpose(warm2[:, :], ident[:, :], ident[:, :])
    xTs = []
    for b in range(BPC):
        sd = small.tile([128, 1], F32, tag="sd", name="sd")
        nc.scalar.activation(out=sd, in_=mvs[b][:, 1:2], func=AF.Sqrt,
                             scale=sqrt_scale)
        nc.vector.tensor_scalar_add(out=sd, in0=sd, scalar1=eps_s)
        rstd = small.tile([128, 1], F32, tag="rstd", name="rstd")
        nc.vector.reciprocal(rstd, sd)
        xhat = sb.tile([128, D], BF16, tag=f"xhat{b}", name=f"xhat{b}")
        nc.vector.tensor_scalar(out=xhat, in0=Osws[b], scalar1=mvs[b][:, 0:1],
                                scalar2=rstd, op0=ALU.subtract, op1=ALU.mult)
        xT = sb.tile([128, 4, 128], BF16, tag=f"xT{b}", name=f"xT{b}")
        for dc in range(4):
            tp = ps_xt.tile([128, 128], BF16, tag="xtp", name="xtp")
            nc.tensor.transpose(tp[:, :], xhat[:, dc * 128:(dc + 1) * 128],
                                ident[:, :])
            nc.scalar.copy(out=xT[:, dc, :], in_=tp[:, :])
        xTs.append(xT)
        if dbg:
            nc.gpsimd.dma_start(out=io["dbg_xhat"][b], in_=xhat[:, :])

    if PH < 5:
        for b in range(BPC):
            nc.vector.memset(out_sb[b], 0.0)
            eng = nc.sync if b == 0 else nc.scalar
            eng.dma_start(out=io["out"][b, :, :], in_=out_sb[b][:, :])
        return

    # ---- phase D: fc + residual + BN partial sums
    junk = sb.tile([128, 512], BF16, tag="junk", name="junk")
    for b in range(BPC):
        for pt in range(NPT):
            O2 = ps_fc.tile([128, 512], F32, tag="O2", name="O2")
            for dc in range(4):
                nc.tensor.matmul(O2[:, :], xTs[b][:, dc, :], wfc_sb[:, pt, dc, :],
                                 start=dc == 0, stop=dc == 3)
            seg = out_sb[b][:, pt * 512:(pt + 1) * 512]
            i = b * NPT + pt
            nc.vector.scalar_tensor_tensor(
                out=seg, in0=O2[:, :], scalar=1.0,
                in1=veff_sb[b][:, pt * 512:(pt + 1) * 512],
                op0=ALU.mult, op1=ALU.add, accum_out=pcols[:, i:i + 1])
            nc.scalar.activation(out=junk, in_=seg, func=AF.Square,
                                 accum_out=pcols[:, 16 + i:17 + i])

    # ---- phase E: BN stats AllReduce + normalize + store
    stats2 = stat.tile([128, 2], F32, tag="stats2", name="stats2")
    nc.vector.reduce_sum(stats2[:, 0:1], pcols[:, 0:16], axis=AX.X)
    nc.vector.reduce_sum(stats2[:, 1:2], pcols[:, 16:32], axis=AX.X)

    cin = dram.tile([128, 2], F32, tag="cin", name="cin")
    cout = dram.tile([128, 2], F32, tag="cout", name="cout")
    nc.gpsimd.dma_start(out=cin[:, :], in_=stats2)
    if os.environ.get("BASS_SKIP_COLL", "0") == "1":
        nc.gpsimd.dma_start(out=cout[:, :], in_=cin[:, :])
    else:
        nc.gpsimd.collective_compute(
            "AllReduce",
            mybir.AluOpType.add,
            replica_groups=[list(range(N_CORES))],
            ins=[cin.opt()],
            outs=[cout.opt()],
        )
    red = stat.tile([128, 2], F32, tag="red", name="red")
    nc.gpsimd.dma_start(out=red[:, :], in_=cout[:, :])

    inv_n = 1.0 / float(B * P)
    mean = small.tile([128, 1], F32, tag="mean", name="mean")
    nc.vector.tensor_scalar_mul(out=mean, in0=red[:, 0:1], scalar1=inv_n)
    ex2 = small.tile([128, 1], F32, tag="ex2", name="ex2")
    nc.vector.tensor_scalar_mul(out=ex2, in0=red[:, 1:2], scalar1=inv_n)
    msq = small.tile([128, 1], F32, tag="msq", name="msq")
    nc.vector.tensor_mul(out=msq, in0=mean, in1=mean)
    var = small.tile([128, 1], F32, tag="var", name="var")
    nc.vector.tensor_sub(out=var, in0=ex2, in1=msq)
    sdv = small.tile([128, 1], F32, tag="sdv", name="sdv")
    nc.scalar.activation(out=sdv, in_=var, func=AF.Sqrt, bias=epsbn)
    invs = small.tile([128, 1], F32, tag="invs", name="invs")
    nc.vector.reciprocal(invs, sdv)
    scl = small.tile([128, 1], F32, tag="scl", name="scl")
    nc.vector.tensor_mul(out=scl, in0=bng, in1=invs)
    tmp = small.tile([128, 1], F32, tag="tmp", name="tmp")
    nc.vector.tensor_mul(out=tmp, in0=mean, in1=scl)
    shf = small.tile([128, 1], F32, tag="shf", name="shf")
    nc.vector.tensor_sub(out=shf, in0=bnb, in1=tmp)

    for b in range(BPC):
        nc.vector.tensor_scalar(out=out_sb[b][:, :], in0=out_sb[b][:, :],
                                scalar1=scl, scalar2=shf,
                                op0=ALU.mult, op1=ALU.add)
        eng = nc.sync if b == 0 else nc.scalar
        eng.dma_start(out=io["out"][b, :, :], in_=out_sb[b][:, :])


def _build(scales):
    key = (os.environ.get("BASS_SKIP_COLL", "0"), W8MODE,
           os.environ.get("BASS_PHASES", "9"),
           os.environ.get("BASS_DEBUG_DUMP", "0"), tuple(sorted(scales.items())))
    if key in _BUILD_CACHE:
        return _BUILD_CACHE[key]
    nc = bacc.Bacc("TRN2", target_bir_lowering=False, debug=False, num_devices=N_CORES)
    io = {
        "qa": nc.dram_tensor("qa", [4, 128, NCH // 4, 2, 128], BF16, kind="ExternalInput").ap(),
        "ka": nc.dram_tensor("ka", [4, 128, NCH // 4, 2, 128], BF16, kind="ExternalInput").ap(),
        "va": nc.dram_tensor("va", [4, 128, NCH // 4, 2, 128], BF16, kind="ExternalInput").ap(),
        "veff": nc.dram_tensor("veff", [BPC, C, P], BF16, kind="ExternalInput").ap(),
        "wq": _wtensor(nc, "wq", [4, 128, NCH // 4, 4, 128]),
        "wk": _wtensor(nc, "wk", [4, 128, NCH // 4, 4, 128]),
        "wv": _wtensor(nc, "wv", [4, 128, NCH // 4, 512]),
        "wfc": _wtensor(nc, "wfc", [128, NPT, 4, 512]),
        "bng": nc.dram_tensor("bng", [C, 1], F32, kind="ExternalInput").ap(),
        "bnb": nc.dram_tensor("bnb", [C, 1], F32, kind="ExternalInput").ap(),
        "out": nc.dram_tensor("out", [BPC, C, P], BF16, kind="ExternalOutput").ap(),
    }
    if os.environ.get("BASS_DEBUG_DUMP", "0") == "1":
        io.update({
            "dbg_qt": nc.dram_tensor("dbg_qt", [BPC, 128, 512], BF16, kind="ExternalOutput").ap(),
            "dbg_kt": nc.dram_tensor("dbg_kt", [BPC, 128, 512], BF16, kind="ExternalOutput").ap(),
            "dbg_v": nc.dram_tensor("dbg_v", [BPC, 128, 512], BF16, kind="ExternalOutput").ap(),
            "dbg_osc": nc.dram_tensor("dbg_osc", [BPC, 128, 512], F32, kind="ExternalOutput").ap(),
            "dbg_xhat": nc.dram_tensor("dbg_xhat", [BPC, 128, 512], BF16, kind="ExternalOutput").ap(),
        })
    from contextlib import ExitStack
    with tile.TileContext(nc) as tc, ExitStack() as ctx:
        _emit(ctx, nc, tc, io, scales)
    nc.compile()
    _BUILD_CACHE[key] = nc
    return nc


def _pow2_scale(w):
    m = float(np.abs(w).max())
    return float(2.0 ** np.floor(np.log2(FP8_MAX_TARGET / m)))


def _wtensor(nc, name, shape):
    if W8MODE == "bf16":
        return nc.dram_tensor(name, shape, mybir.dt.bfloat16,
                              kind="ExternalInput").ap()
    return nc.dram_tensor(name, shape, mybir.dt.uint8,
                          kind="ExternalInput").bitcast(FP8).ap()


def _q8(w, s):
    w = np.asarray(w, np.float32) * s
    if W8MODE == "bf16":
        return np.ascontiguousarray(w.astype(ml_dtypes.bfloat16))
    dt8 = ml_dtypes.float8_e3m4 if W8MODE == "e3" else ml_dtypes.float8_e4m3
    return np.ascontiguousarray(w.astype(dt8)).view(np.uint8)


def _bf16(x):
    return np.ascontiguousarray(np.asarray(x, np.float32).astype(ml_dtypes.bfloat16))


def _pack_acts(xf):
    # [b, c, p] f32 -> [128, NCH, b, c] bf16  (pixel-in-chunk, chunk, batch, channel)
    b = xf.shape[0]
    return _bf16(xf.transpose(2, 0, 1).reshape(NCH, 128, b, C).transpose(1, 0, 2, 3))


def _qmajor(x):
    # [128, NCH, ...] -> [4, 128, NCH//4, ...] (contiguous per-quarter DMA)
    s = x.shape
    return np.ascontiguousarray(
        x.reshape(128, 4, NCH // 4, *s[2:]).transpose(1, 0, 2, *range(3, x.ndim + 1)))


def kernel(v, k, q, w_qs, w_ks, w_vs, w_fc, ln_gamma, ln_beta, temperature,
           bn_gamma, bn_beta, **_ignored):
    v = np.asarray(v, np.float32)
    k = np.asarray(k, np.float32)
    q = np.asarray(q, np.float32)
    w_qs = np.asarray(w_qs, np.float32)
    w_ks = np.asarray(w_ks, np.float32)
    w_vs = np.asarray(w_vs, np.float32)
    w_fc = np.asarray(w_fc, np.float32)
    ln_gamma = np.asarray(ln_gamma, np.float32)
    ln_beta = np.asarray(ln_beta, np.float32)
    temp = float(np.asarray(temperature))
    bn_gamma = np.asarray(bn_gamma, np.float32)
    bn_beta = np.asarray(bn_beta, np.float32)

    qf = q.reshape(B, C, P)
    kf = k.reshape(B, C, P)
    vf = v.reshape(B, C, P)
    qa = _pack_acts(qf)   # [32, 128, 16, 128]
    ka = _pack_acts(kf)
    va = _pack_acts(vf)

    wqT = (w_qs / temp).T            # [P, D]
    wkT = w_ks.T
    wvT = w_vs.T
    wfcT_eff = (w_fc * ln_gamma[None, :]).T   # [D, P]
    s_q = _pow2_scale(wqT)
    s_k = _pow2_scale(wkT)
    s_v = _pow2_scale(wvT)
    s_fc = _pow2_scale(wfcT_eff)
    scales = {"s_q": s_q, "s_k": s_k, "s_v": s_v, "s_fc": s_fc}

    wq = _qmajor(_q8(wqT.reshape(NCH, 128, 4, 128).transpose(1, 0, 2, 3), s_q))
    wk = _qmajor(_q8(wkT.reshape(NCH, 128, 4, 128).transpose(1, 0, 2, 3), s_k))
    wv = _qmajor(_q8(wvT.reshape(NCH, 128, 512).transpose(1, 0, 2), s_v))
    wfc = _q8(wfcT_eff.reshape(4, 128, NPT, 512).transpose(1, 2, 0, 3), s_fc)
    bias_fc = (w_fc @ ln_beta).astype(np.float32)
    veff = vf + bias_fc[None, None, :]
    bng = np.ascontiguousarray(bn_gamma.reshape(C, 1))
    bnb = np.ascontiguousarray(bn_beta.reshape(C, 1))

    nc = _build(scales)
    in_maps = []
    for i in range(N_CORES):
        bs = slice(BPC * i, BPC * (i + 1))
        in_maps.append({
            "qa": _qmajor(qa[:, :, bs, :]),
            "ka": _qmajor(ka[:, :, bs, :]),
            "va": _qmajor(va[:, :, bs, :]),
            "veff": _bf16(veff[bs]),
            "wq": wq, "wk": wk, "wv": wv, "wfc": wfc,
            "bng": bng, "bnb": bnb,
        })
    res = run_bass_kernel_spmd(nc, in_maps, core_ids=list(range(N_CORES)))
    global LAST_RESULTS
    LAST_RESULTS = res
    out = np.concatenate([np.asarray(res.results[i]["out"], dtype=np.float32)
                          for i in range(N_CORES)], axis=0)
    return out.reshape(B, C, HH, WW)


MODE = f"v2-{W8MODE}w-bf16a"


# revision 43
# speedup vs baseline: 1.3452x; 1.3452x over previous
"""Trainium2 Bass kernel for nn_MultiHeadAttention (channel-attention transformer block).

Math (per batch b, with X* = reshape(*, [C, P]), P = 4096, C = 128, D = 512):
  Q = Xq @ (Wq/temp)^T, K = Xk @ Wk^T, V = Xv @ Wv^T            [C, D]
  per head h (8 heads, ld=64): A_h = softmax(Q_h K_h^T); O_h = A_h V_h
  O = silu(O); O = (O - mean)/(unbiased_std + eps)   (LN affine folded into fc)
  out_pre = (v + Wfc@ln_beta) + O @ (Wfc*ln_gamma)^T
  out = BatchNorm2d(out_pre)   (batch stats over (b,h,w), biased var)

Sharding: data-parallel over batch, 2 batches per core on 8 cores; BatchNorm
statistics combined with a tiny AllReduce ([128,2] per core).

Design notes:
  - weights quantized to fp8 E3M4 with power-of-2 scales folded into existing
    per-row scalars (exp scale, sigmoid scale, LN-sqrt scale) => zero extra ops
  - activations/outputs in bf16 (DMA ~19MB/core vs 50MB f32); everything is
    prefetched to SBUF via contiguous quarter-major DMAs
  - Q/K projections weight-stationary with both batches concatenated into one
    256-wide moving operand => QT/KT land pre-transposed, one PSUM group/bank
  - attention via the ST trick: ST_h = K_h Q_h^T, exp(ST) = A^T is directly
    the AV stationary operand (no PE transposes / PSUM shuffles); softmax
    denominators via a 1-wide matmul against a ones vector
  - ACT table grouping (all exps, then sigmoids, then sqrts)
  - BN partial sums via accum_out on the residual-add (DVE) + Square (ACT)
  - bf16 output, upcast on host; BN stats via one [128,2] AllReduce

HW-crash landmine (cost a lot of bisection): matmul/transpose outputs into
PSUM must be plain 2D slices of tiles that fit ONE 2KB bank, and a psum tile
may hold only one OPEN accumulation group at a time. 3D-sliced outs
(tile[:, h, :]) or multi-bank out tiles crash the device (NRT UNRECOVERABLE).
"""

import os

import numpy as np
import ml_dtypes

import concourse.mybir as mybir
import concourse.tile as tile
from concourse import bacc
from concourse.bass_utils import run_bass_kernel_spmd
from concourse.masks import make_identity

# ---- problem constants (hardcoded per contract) ----
B, C, HH, WW = 16, 128, 64, 64
P = HH * WW           # 4096
NH, LD = 8, 64
D = NH * LD           # 512
N_CORES = 8
BPC = B // N_CORES    # 2 batches per core
NCH = P // 128        # 32 pixel chunks (contraction)
NPT = P // 512        # 8 output column tiles for fc
LN_EPS = 1e-6
BN_EPS = 1e-5
F32 = mybir.dt.float32
BF16 = mybir.dt.bfloat16
W8MODE = os.environ.get("BASS_W8", "e3")  # e3 | e4 | bf16
FP8 = {"e3": mybir.dt.float8e3, "e4": mybir.dt.float8e4,
       "bf16": mybir.dt.bfloat16}[W8MODE]
W8BYTES = 2 if W8MODE == "bf16" else 1
FP8_MAX_TARGET = {"e3": 14.0, "e4": 224.0, "bf16": 14.0}[W8MODE]

_BUILD_CACHE: dict = {}
LAST_RESULTS = None  # BassKernelResults of the most recent run (for profiling)

# host-side fp8 scales (power of two), computed at pack time, baked into build
_SCALES: dict = {}


def _emit(ctx, nc, tc, io, scales):
    PH = int(os.environ.get("BASS_PHASES", "9"))
    AF = mybir.ActivationFunctionType
    ALU = mybir.AluOpType
    AX = mybir.AxisListType
    s_q, s_k, s_v, s_fc = (scales[k] for k in ("s_q", "s_k", "s_v", "s_fc"))

    consts = ctx.enter_context(tc.tile_pool(name="consts", bufs=1))
    big = ctx.enter_context(tc.tile_pool(name="big", bufs=1))
    sb = ctx.enter_context(tc.tile_pool(name="sb", bufs=2))
    small = ctx.enter_context(tc.tile_pool(name="small", bufs=4))
    stat = ctx.enter_context(tc.tile_pool(name="stat", bufs=1))
    dram = ctx.enter_context(tc.tile_pool(name="dram", bufs=1, space="DRAM"))

    ident = consts.tile([128, 128], BF16, tag="ident", name="ident")
    identf = consts.tile([128, 128], F32, tag="identf", name="identf")
    make_identity(nc, identf)
    nc.vector.tensor_copy(out=ident, in_=identf)
    ones = consts.tile([128, 1], BF16, tag="ones", name="ones")
    nc.vector.memset(ones, 1.0)

    bng = consts.tile([128, 1], F32, tag="bng", name="bng")
    bnb = consts.tile([128, 1], F32, tag="bnb", name="bnb")
    epsbn = consts.tile([128, 1], F32, tag="epsbn", name="epsbn")
    nc.gpsimd.dma_start(out=bng, in_=io["bng"][:, :])
    nc.gpsimd.dma_start(out=bnb, in_=io["bnb"][:, :])
    nc.vector.memset(epsbn, BN_EPS)

    # ---- prefetch: everything lives in SBUF (quartered DMAs so compute can
    # start as soon as the first chunks land)
    qa_sb = big.tile([128, NCH, 2, 128], BF16, tag="qa_sb", name="qa_sb")
    ka_sb = big.tile([128, NCH, 2, 128], BF16, tag="ka_sb", name="ka_sb")
    va_sb = big.tile([128, NCH, 2, 128], BF16, tag="va_sb", name="va_sb")
    wq_sb = big.tile([128, NCH, 4, 128], FP8, tag="wq_sb", name="wq_sb")
    wk_sb = big.tile([128, NCH, 4, 128], FP8, tag="wk_sb", name="wk_sb")
    wv_sb = big.tile([128, NCH, 512], FP8, tag="wv_sb", name="wv_sb")
    QTR = NCH // 4
    for q4 in range(4):
        cs = slice(QTR * q4, QTR * (q4 + 1))
        nc.sync.dma_start(out=qa_sb[:, cs, :, :], in_=io["qa"][q4])
        nc.scalar.dma_start(out=wq_sb[:, cs, :, :], in_=io["wq"][q4])
    for q4 in range(4):
        cs = slice(QTR * q4, QTR * (q4 + 1))
        nc.sync.dma_start(out=ka_sb[:, cs, :, :], in_=io["ka"][q4])
        nc.scalar.dma_start(out=wk_sb[:, cs, :, :], in_=io["wk"][q4])
    for q4 in range(4):
        cs = slice(QTR * q4, QTR * (q4 + 1))
        nc.sync.dma_start(out=va_sb[:, cs, :, :], in_=io["va"][q4])
        nc.scalar.dma_start(out=wv_sb[:, cs, :], in_=io["wv"][q4])

    # residual (+ folded fc bias) and fc weights queue BEHIND the phase-A
    # feeds, split evenly across both HWDGE queues so neither runs long
    veff_sb = []
    for b in range(BPC):
        t = big.tile([128, P], BF16, tag=f"veff{b}", name=f"veff{b}")
        eng = nc.sync if b == 0 else nc.scalar
        eng.dma_start(out=t, in_=io["veff"][b, :, :])
        veff_sb.append(t)
    wfc_sb = big.tile([128, NPT, 4, 512], FP8, tag="wfc_sb", name="wfc_sb")
    nc.sync.dma_start(out=wfc_sb[:, 0:4, :, :], in_=io["wfc"][:, 0:4, :, :])
    nc.scalar.dma_start(out=wfc_sb[:, 4:8, :, :], in_=io["wfc"][:, 4:8, :, :])
    out_sb = [big.tile([128, P], BF16, tag=f"outb{b}", name=f"outb{b}")
              for b in range(BPC)]

    # ---- phase A: QKV projections, accumulating over the P=4096 contraction.
    # Q,K weight-stationary with both batches as one 256-wide moving operand
    # (outputs arrive transposed [d, b|c]); V activation-stationary 512-wide.
    ctx_a1 = tc.tile_pool(name="ps_qk", bufs=3, space="PSUM")
    ps_qk = ctx_a1.__enter__()
    ctx_a2 = tc.tile_pool(name="ps_v", bufs=1, space="PSUM")
    ps_v = ctx_a2.__enter__()
    warm = ps_v.tile([128, 128], BF16, tag="warm", name="warm")
    for _ in range(40):
        nc.tensor.transpose(warm[:, :], ident[:, :], ident[:, :])

    qkv_sb = []
    for b in range(BPC):
        QT_sb = sb.tile([128, 512], BF16, tag=f"QT_sb{b}", name=f"QT_sb{b}")
        KT_sb = sb.tile([128, 512], BF16, tag=f"KT_sb{b}", name=f"KT_sb{b}")
        V_sb = sb.tile([128, 512], BF16, tag=f"V_sb{b}", name=f"V_sb{b}")
        qkv_sb.append((QT_sb, KT_sb, V_sb))

    if PH >= 2:
        for w_sb, a_sb, which in ((wq_sb, qa_sb, 0), (wk_sb, ka_sb, 1)):
            for dc in range(4):
                pq = ps_qk.tile([128, 256], F32, tag="qkp", name="qkp")
                for chunk in range(NCH):
                    nc.tensor.matmul(pq[:, :], w_sb[:, chunk, dc, :],
                                     a_sb[:, chunk, :, :],
                                     start=chunk == 0, stop=chunk == NCH - 1)
                fo = dc * 128
                dst0 = qkv_sb[0][which]
                dst1 = qkv_sb[1][which]
                nc.scalar.copy(out=dst0[:, fo:fo + 128], in_=pq[:, 0:128])
                nc.scalar.copy(out=dst1[:, fo:fo + 128], in_=pq[:, 128:256])
        for b in range(BPC):
            vp = ps_v.tile([128, 512], F32, tag=f"vp{b}", name=f"vp{b}")
            for chunk in range(NCH):
                nc.tensor.matmul(vp[:, :], va_sb[:, chunk, b, :],
                                 wv_sb[:, chunk, :],
                                 start=chunk == 0, stop=chunk == NCH - 1)
            nc.scalar.copy(out=qkv_sb[b][2], in_=vp[:, :])
    ctx_a2.__exit__(None, None, None)
    ctx_a1.__exit__(None, None, None)

    ps_s = ctx.enter_context(tc.tile_pool(name="ps_s", bufs=2, space="PSUM"))
    ps_xt = ctx.enter_context(tc.tile_pool(name="ps_xt", bufs=1, space="PSUM"))
    ps_o = ctx.enter_context(tc.tile_pool(name="ps_o", bufs=1, space="PSUM"))
    ps_fc = ctx.enter_context(tc.tile_pool(name="ps_fc", bufs=2, space="PSUM"))

    # per-channel partial sums: cols 0..15 = sum(out) per (b,pt), 16..31 = sum(out^2)
    pcols = stat.tile([128, 32], F32, tag="pcols", name="pcols")

    exp_scale = 1.0 / (s_q * s_k)
    sig_scale = 1.0 / s_v
    sqrt_scale = (float(D) / (D - 1)) * s_fc * s_fc
    eps_s = LN_EPS * s_v * s_fc

    dbg = os.environ.get("BASS_DEBUG_DUMP", "0") == "1" and "dbg_qt" in io
    if dbg:
        for b in range(BPC):
            nc.gpsimd.dma_start(out=io["dbg_qt"][b], in_=qkv_sb[b][0][:, :])
            nc.gpsimd.dma_start(out=io["dbg_kt"][b], in_=qkv_sb[b][1][:, :])
            nc.gpsimd.dma_start(out=io["dbg_v"][b], in_=qkv_sb[b][2][:, :])

    if PH < 3:
        for b in range(BPC):
            nc.vector.memset(out_sb[b], 0.0)
            eng = nc.sync if b == 0 else nc.scalar
            eng.dma_start(out=io["out"][b, :, :], in_=out_sb[b][:, :])
        return

    # ---- phase B: attention, ST formulation. ST_h = K_h Q_h^T comes out
    # [e, c]; exp(ST) is A^T which is exactly the AV stationary operand, so no
    # PE transposes or PSUM->SBUF shuffles are needed. Softmax denominators
    # come from a 1-wide matmul against a ones vector (same stationary).
    # All exps are contiguous so the ACT Exp table loads once.
    ls = ps_o.tile([128, 16], F32, tag="ls", name="ls")
    warm2 = ps_xt.tile([128, 128], BF16, tag="xtp", name="warm2")
    eftp = ctx.enter_context(tc.tile_pool(name="eftp", bufs=16))
    Ops = []
    efts = {}
    for b in range(BPC):
        QT_sb, KT_sb, V_sb = qkv_sb[b]
        for h in range(NH):
            po = (h % 2) * 64
            fo = (h // 2) * 128
            ST = ps_s.tile([128, 128], F32, tag="S", name="S")
            nc.tensor.matmul(ST[:, :], KT_sb[po:po + 64, fo:fo + 128],
                             QT_sb[po:po + 64, fo:fo + 128], start=True, stop=True)
            eft = eftp.tile([128, 128], BF16, tag="eft", name="eft")
            nc.scalar.activation(out=eft, in_=ST[:, :], func=AF.Exp,
                                 scale=exp_scale)
            efts[(b, h)] = eft
    Oscs = []
    for b in range(BPC):
        V_sb = qkv_sb[b][2]
        Opsum = ps_o.tile([128, 512], F32, tag=f"O{b}", name=f"O{b}")
        Ops.append(Opsum)
        for h in range(NH):
            nc.tensor.matmul(Opsum[:, h * 64:(h + 1) * 64], efts[(b, h)][:, :],
                             V_sb[:, h * 64:(h + 1) * 64], start=True, stop=True)
            nc.tensor.matmul(ls[:, b * 8 + h:b * 8 + h + 1], efts[(b, h)][:, :],
                             ones[:, :], start=True, stop=True)

    for b in range(BPC):
        rs = small.tile([128, 8], F32, tag="rs", name="rs")
        nc.vector.reciprocal(rs, ls[:, b * 8:b * 8 + 8])
        Osc = sb.tile([128, 512], F32, tag=f"Osc{b}", name=f"Osc{b}")
        for h in range(NH):
            nc.scalar.activation(out=Osc[:, h * 64:(h + 1) * 64],
                                 in_=Ops[b][:, h * 64:(h + 1) * 64],
                                 func=AF.Copy, scale=rs[:, h:h + 1])
        Oscs.append(Osc)
        if dbg:
            nc.gpsimd.dma_start(out=io["dbg_osc"][b], in_=Osc[:, :])

    if PH < 4:
        for b in range(BPC):
            nc.vector.memset(out_sb[b], 0.0)
            eng = nc.sync if b == 0 else nc.scalar
            eng.dma_start(out=io["out"][b, :, :], in_=out_sb[b][:, :])
        return

    # ---- phase C: silu + LN (sigmoids grouped, then sqrts, for table reuse)
    Osws = []
    mvs = []
    for b in range(BPC):
        sg = sb.tile([128, D], F32, tag=f"sg{b}", name=f"sg{b}")
        nc.scalar.activation(out=sg, in_=Oscs[b], func=AF.Sigmoid, scale=sig_scale)
        Osw = sb.tile([128, D], F32, tag=f"Osw{b}", name=f"Osw{b}")
        nc.gpsimd.tensor_mul(out=Osw, in0=Oscs[b], in1=sg)
        st6 = small.tile([128, 6], F32, tag="st6", name="st6")
        nc.vector.bn_stats(out=st6, in_=Osw)
        mv = small.tile([128, 2], F32, tag=f"mv{b}", name=f"mv{b}")
        nc.vector.bn_aggr(out=mv, in_=st6)
        Osws.append(Osw)
        mvs.append(mv)
    for _ in range(24):
        nc.tensor.transpose(warm2[:, :], ident[:, :], ident[:, :])
    xTs = []
    for b in range(BPC):
        sd = small.tile([128, 1], F32, tag="sd", name="sd")
        nc.scalar.activation(out=sd, in_=mvs[b][:, 1:2], func=AF.Sqrt,
                             scale=sqrt_scale)
        nc.vector.tensor_scalar_add(out=sd, in0=sd, scalar1=eps_s)
        rstd = small.tile([128, 1], F32, tag="rstd", name="rstd")
        nc.vector.reciprocal(rstd, sd)
        xhat = sb.tile([128, D], BF16, tag=f"xhat{b}", name=f"xhat{b}")
        nc.vector.tensor_scalar(out=xhat, in0=Osws[b], scalar1=mvs[b][:, 0:1],
                                scalar2=rstd, op0=ALU.subtract, op1=ALU.mult)
        xT = sb.tile([128, 4, 128], BF16, tag=f"xT{b}", name=f"xT{b}")
        for dc in range(4):
            tp = ps_xt.tile([128, 128], BF16, tag="xtp", name="xtp")
            nc.tensor.transpose(tp[:, :], xhat[:, dc * 128:(dc + 1) * 128],
                                ident[:, :])
            nc.scalar.copy(out=xT[:, dc, :], in_=tp[:, :])
        xTs.append(xT)
        if dbg:
            nc.gpsimd.dma_start(out=io["dbg_xhat"][b], in_=xhat[:, :])

    if PH < 5:
        for b in range(BPC):
            nc.vector.memset(out_sb[b], 0.0)
            eng = nc.sync if b == 0 else nc.scalar
            eng.dma_start(out=io["out"][b, :, :], in_=out_sb[b][:, :])
        return

    # ---- phase D: fc + residual + BN partial sums
    junk = sb.tile([128, 512], BF16, tag="junk", name="junk")
    for b in range(BPC):
        for pt in range(NPT):
            O2 = ps_fc.tile([128, 512], F32, tag="O2", name="O2")
            for dc in range(4):
                nc.tensor.matmul(O2[:, :], xTs[b][:, dc, :], wfc_sb[:, pt, dc, :],
                                 start=dc == 0, stop=dc == 3)
            seg = out_sb[b][:, pt * 512:(pt + 1) * 512]
            i = b * NPT + pt
            nc.vector.scalar_tensor_tensor(
                out=seg, in0=O2[:, :], scalar=1.0,
                in1=veff_sb[b][:, pt * 512:(pt + 1) * 512],
                op0=ALU.mult, op1=ALU.add, accum_out=pcols[:, i:i + 1])
            nc.scalar.activation(out=junk, in_=seg, func=AF.Square,
                                 accum_out=pcols[:, 16 + i:17 + i])

    # ---- phase E: BN stats AllReduce + normalize + store
    stats2 = stat.tile([128, 2], F32, tag="stats2", name="stats2")
    nc.vector.reduce_sum(stats2[:, 0:1], pcols[:, 0:16], axis=AX.X)
    nc.vector.reduce_sum(stats2[:, 1:2], pcols[:, 16:32], axis=AX.X)

    cin = dram.tile([128, 2], F32, tag="cin", name="cin")
    cout = dram.tile([128, 2], F32, tag="cout", name="cout")
    nc.gpsimd.dma_start(out=cin[:, :], in_=stats2)
    if os.environ.get("BASS_SKIP_COLL", "0") == "1":
        nc.gpsimd.dma_start(out=cout[:, :], in_=cin[:, :])
    else:
        nc.gpsimd.collective_compute(
            "AllReduce",
            mybir.AluOpType.add,
            replica_groups=[list(range(N_CORES))],
            ins=[cin.opt()],
            outs=[cout.opt()],
        )
    red = stat.tile([128, 2], F32, tag="red", name="red")
    nc.gpsimd.dma_start(out=red[:, :], in_=cout[:, :])

    inv_n = 1.0 / float(B * P)
    mean = small.tile([128, 1], F32, tag="mean", name="mean")
    nc.vector.tensor_scalar_mul(out=mean, in0=red[:, 0:1], scalar1=inv_n)
    ex2 = small.tile([128, 1], F32, tag="ex2", name="ex2")
    nc.vector.tensor_scalar_mul(out=ex2, in0=red[:, 1:2], scalar1=inv_n)
    msq = small.tile([128, 1], F32, tag="msq", name="msq")
    nc.vector.tensor_mul(out=msq, in0=mean, in1=mean)
    var = small.tile([128, 1], F32, tag="var", name="var")
    nc.vector.tensor_sub(out=var, in0=ex2, in1=msq)
    sdv = small.tile([128, 1], F32, tag="sdv", name="sdv")
    nc.scalar.activation(out=sdv, in_=var, func=AF.Sqrt, bias=epsbn)
    invs = small.tile([128, 1], F32, tag="invs", name="invs")
    nc.vector.reciprocal(invs, sdv)
    scl = small.tile([128, 1], F32, tag="scl", name="scl")
    nc.vector.tensor_mul(out=scl, in0=bng, in1=invs)
    tmp = small.tile([128, 1], F32, tag="tmp", name="tmp")
    nc.vector.tensor_mul(out=tmp, in0=mean, in1=scl)
    shf = small.tile([128, 1], F32, tag="shf", name="shf")
    nc.vector.tensor_sub(out=shf, in0=bnb, in1=tmp)

    for b in range(BPC):
        nc.vector.tensor_scalar(out=out_sb[b][:, :], in0=out_sb[b][:, :],
                                scalar1=scl, scalar2=shf,
                                op0=ALU.mult, op1=ALU.add)
        eng = nc.sync if b == 0 else nc.scalar
        eng.dma_start(out=io["out"][b, :, :], in_=out_sb[b][:, :])


def _build(scales):
    key = (os.environ.get("BASS_SKIP_COLL", "0"), W8MODE,
           os.environ.get("BASS_PHASES", "9"),
           os.environ.get("BASS_DEBUG_DUMP", "0"), tuple(sorted(scales.items())))
    if key in _BUILD_CACHE:
        return _BUILD_CACHE[key]
    nc = bacc.Bacc("TRN2", target_bir_lowering=False, debug=False, num_devices=N_CORES)
    io = {
        "qa": nc.dram_tensor("qa", [4, 128, NCH // 4, 2, 128], BF16, kind="ExternalInput").ap(),
        "ka": nc.dram_tensor("ka", [4, 128, NCH // 4, 2, 128], BF16, kind="ExternalInput").ap(),
        "va": nc.dram_tensor("va", [4, 128, NCH // 4, 2, 128], BF16, kind="ExternalInput").ap(),
        "veff": nc.dram_tensor("veff", [BPC, C, P], BF16, kind="ExternalInput").ap(),
        "wq": _wtensor(nc, "wq", [4, 128, NCH // 4, 4, 128]),
        "wk": _wtensor(nc, "wk", [4, 128, NCH // 4, 4, 128]),
        "wv": _wtensor(nc, "wv", [4, 128, NCH // 4, 512]),
        "wfc": _wtensor(nc, "wfc", [128, NPT, 4, 512]),
        "bng": nc.dram_tensor("bng", [C, 1], F32, kind="ExternalInput").ap(),
        "bnb": nc.dram_tensor("bnb", [C, 1], F32, kind="ExternalInput").ap(),
        "out": nc.dram_tensor("out", [BPC, C, P], BF16, kind="ExternalOutput").ap(),
    }
    if os.environ.get("BASS_DEBUG_DUMP", "0") == "1":
        io.update({
            "dbg_qt": nc.dram_tensor("dbg_qt", [BPC, 128, 512], BF16, kind="ExternalOutput").ap(),
            "dbg_kt": nc.dram_tensor("dbg_kt", [BPC, 128, 512], BF16, kind="ExternalOutput").ap(),
            "dbg_v": nc.dram_tensor("dbg_v", [BPC, 128, 512], BF16, kind="ExternalOutput").ap(),
            "dbg_osc": nc.dram_tensor("dbg_osc", [BPC, 128, 512], F32, kind="ExternalOutput").ap(),
            "dbg_xhat": nc.dram_tensor("dbg_xhat", [BPC, 128, 512], BF16, kind="ExternalOutput").ap(),
        })
    from contextlib import ExitStack
    with tile.TileContext(nc) as tc, ExitStack() as ctx:
        _emit(ctx, nc, tc, io, scales)
    nc.compile()
    _BUILD_CACHE[key] = nc
    return nc


def _pow2_scale(w):
    m = float(np.abs(w).max())
    return float(2.0 ** np.floor(np.log2(FP8_MAX_TARGET / m)))


def _wtensor(nc, name, shape):
    if W8MODE == "bf16":
        return nc.dram_tensor(name, shape, mybir.dt.bfloat16,
                              kind="ExternalInput").ap()
    return nc.dram_tensor(name, shape, mybir.dt.uint8,
                          kind="ExternalInput").bitcast(FP8).ap()


def _q8(w, s):
    w = np.asarray(w, np.float32) * s
    if W8MODE == "bf16":
        return np.ascontiguousarray(w.astype(ml_dtypes.bfloat16))
    dt8 = ml_dtypes.float8_e3m4 if W8MODE == "e3" else ml_dtypes.float8_e4m3
    return np.ascontiguousarray(w.astype(dt8)).view(np.uint8)


def _bf16(x):
    return np.ascontiguousarray(np.asarray(x, np.float32).astype(ml_dtypes.bfloat16))


def _pack_acts(xf):
    # [b, c, p] f32 -> [128, NCH, b, c] bf16  (pixel-in-chunk, chunk, batch, channel)
    b = xf.shape[0]
    return _bf16(xf.transpose(2, 0, 1).reshape(NCH, 128, b, C).transpose(1, 0, 2, 3))


def _qmajor(x):
    # [128, NCH, ...] -> [4, 128, NCH//4, ...] (contiguous per-quarter DMA)
    s = x.shape
    return np.ascontiguousarray(
        x.reshape(128, 4, NCH // 4, *s[2:]).transpose(1, 0, 2, *range(3, x.ndim + 1)))


def kernel(v, k, q, w_qs, w_ks, w_vs, w_fc, ln_gamma, ln_beta, temperature,
           bn_gamma, bn_beta, **_ignored):
    v = np.asarray(v, np.float32)
    k = np.asarray(k, np.float32)
    q = np.asarray(q, np.float32)
    w_qs = np.asarray(w_qs, np.float32)
    w_ks = np.asarray(w_ks, np.float32)
    w_vs = np.asarray(w_vs, np.float32)
    w_fc = np.asarray(w_fc, np.float32)
    ln_gamma = np.asarray(ln_gamma, np.float32)
    ln_beta = np.asarray(ln_beta, np.float32)
    temp = float(np.asarray(temperature))
    bn_gamma = np.asarray(bn_gamma, np.float32)
    bn_beta = np.asarray(bn_beta, np.float32)

    qf = q.reshape(B, C, P)
    kf = k.reshape(B, C, P)
    vf = v.reshape(B, C, P)
    qa = _pack_acts(qf)   # [32, 128, 16, 128]
    ka = _pack_acts(kf)
    va = _pack_acts(vf)

    wqT = (w_qs / temp).T            # [P, D]
    wkT = w_ks.T
    wvT = w_vs.T
    wfcT_eff = (w_fc * ln_gamma[None, :]).T   # [D, P]
    s_q = _pow2_scale(wqT)
    s_k = _pow2_scale(wkT)
    s_v = _pow2_scale(wvT)
    s_fc = _pow2_scale(wfcT_eff)
    scales = {"s_q": s_q, "s_k": s_k, "s_v": s_v, "s_fc": s_fc}

    wq = _qmajor(_q8(wqT.reshape(NCH, 128, 4, 128).transpose(1, 0, 2, 3), s_q))
    wk = _qmajor(_q8(wkT.reshape(NCH, 128, 4, 128).transpose(1, 0, 2, 3), s_k))
    wv = _qmajor(_q8(wvT.reshape(NCH, 128, 512).transpose(1, 0, 2), s_v))
    wfc = _q8(wfcT_eff.reshape(4, 128, NPT, 512).transpose(1, 2, 0, 3), s_fc)
    bias_fc = (w_fc @ ln_beta).astype(np.float32)
    veff = vf + bias_fc[None, None, :]
    bng = np.ascontiguousarray(bn_gamma.reshape(C, 1))
    bnb = np.ascontiguousarray(bn_beta.reshape(C, 1))

    nc = _build(scales)
    in_maps = []
    for i in range(N_CORES):
        bs = slice(BPC * i, BPC * (i + 1))
        in_maps.append({
            "qa": _qmajor(qa[:, :, bs, :]),
            "ka": _qmajor(ka[:, :, bs, :]),
            "va": _qmajor(va[:, :, bs, :]),
            "veff": _bf16(veff[bs]),
            "wq": wq, "wk": wk, "wv": wv, "wfc": wfc,
            "bng": bng, "bnb": bnb,
        })
    res = run_bass_kernel_spmd(nc, in_maps, core_ids=list(range(N_CORES)))
    global LAST_RESULTS
    LAST_RESULTS = res
    out = np.concatenate([np.asarray(res.results[i]["out"], dtype=np.float32)
                          for i in range(N_CORES)], axis=0)
    return out.reshape(B, C, HH, WW)


MODE = f"v2-{W8MODE}w-bf16a"
